# revision 37
# baseline (speedup 1.0000x reference)
"""Trainium2 Bass kernel for nn_AttentionBlock (GroupNorm + FiLM + MHA + proj + residual).

Sharding: data-parallel over batch B=16 across 8 NeuronCores (2 batches/core).
All inputs are taken in full, sharded/laid-out on host, run via
run_bass_kernel_spmd, and the full [16, 512, 32, 32] output is reassembled.

Self-contained: only imports the environment toolchain (concourse/bass_rust).
"""

import os

import numpy as np
import ml_dtypes

import bass_rust
import concourse.bass as bass
import concourse.tile as tile
from concourse import mybir
from concourse.bass_utils import run_bass_kernel_spmd

# ---------------------------------------------------------------- problem dims
B, C, HSP, WSP = 16, 512, 32, 32
T = HSP * WSP            # 1024
E = 1024
NCORES = 8
BLOC = B // NCORES       # 2 batches per core
NG = 32                  # groups
NH = 8                   # heads
CH = C // NH             # 64 head channels
EPS = 1e-5
GSIZE = C // NG          # 16 channels per group
NELEM = GSIZE * T        # elements per group (16384)
SCALE = 1.0 / np.sqrt(np.sqrt(float(CH)))

F32 = mybir.dt.float32
F32R = mybir.dt.float32r
BF16 = mybir.dt.bfloat16

# set by test.py to enable NTFF tracing; harness leaves it off
TRACE = bool(int(os.environ.get("BASS_ATTN_TRACE", "0")))
LAST_EXEC_NS = None
LAST_TRACE_DIR = None


# ------------------------------------------------- sync-wait splitting post-pass
def _split_excess_waits(nc, max_waits=2):
    """walrus in this container encodes at most 2 sem waits per instruction.

    Move excess waits onto freshly inserted NOPs on the same engine right
    before the offending instruction (identical semantics: same point in that
    engine's program order).
    """
    n_id = 0
    for f in nc.m.functions:
        for blk in f.blocks:
            il = blk.instructions  # live list
            idx = 0
            while idx < len(il):
                inst = il[idx]
                si = inst.sync_info
                if si is None or not si.on_wait:
                    idx += 1
                    continue
                budget = 1
                if len(si.on_wait) > budget:
                    waits = list(si.on_wait)
                    keep = waits[-budget:]
                    excess = waits[:-budget]
                    pos = idx
                    for j in range(0, len(excess), 1):
                        grp = excess[j:j + 1]
                        nop = mybir.InstNoOp(name=f"wsplit-{n_id}", ins=[], outs=[])
                        n_id += 1
                        nop.engine = inst.engine
                        nop.sync_info = mybir.SyncInfo(on_wait=grp, on_update=[])
                        try:
                            nc.register_instruction(nop, overwrite=True)
                        except Exception:
                            pass
                        il.insert(pos, nop)
                        pos += 1
                        idx += 1
                    inst.sync_info = mybir.SyncInfo(
                        on_wait=keep, on_update=list(si.on_update or [])
                    )
                idx += 1


def _install_tile_tail_patch():
    return


# ------------------------------------------------------------- device program
def build_program(with_proj_bias: bool, with_qkv_bias: bool = False):
    _install_tile_tail_patch()
    nc = bass.Bass("TRN2", target_bir_lowering=False, debug=False)

    d_x = nc.dram_tensor("x", [BLOC, C, T], F32, kind="ExternalInput").ap()
    d_embT = nc.dram_tensor("embT", [E, BLOC], F32, kind="ExternalInput").ap()
    d_qkvwT = nc.dram_tensor("qkv_wT", [C, 3 * C], BF16, kind="ExternalInput").ap()
    d_embwT = nc.dram_tensor("emb_wT", [E, 2 * C], BF16, kind="ExternalInput").ap()
    d_projwT = nc.dram_tensor("proj_wT", [C, C], BF16, kind="ExternalInput").ap()
    d_gamma = nc.dram_tensor("gamma4", [128, 4], F32, kind="ExternalInput").ap()
    d_beta = nc.dram_tensor("beta4", [128, 4], F32, kind="ExternalInput").ap()
    d_maskg = nc.dram_tensor("mask_g", [128, 8], F32, kind="ExternalInput").ap()
    d_maskb = nc.dram_tensor("mask_b", [8, 128], F32, kind="ExternalInput").ap()
    d_qkvb = nc.dram_tensor("qkvb_qk", [128, 8], F32, kind="ExternalInput").ap()
    d_embb = nc.dram_tensor("emb_b2", [2, 2 * C], F32, kind="ExternalInput").ap()
    d_projb = (nc.dram_tensor("proj_b4", [128, 4], F32, kind="ExternalInput").ap()
               if with_proj_bias else None)
    d_out = nc.dram_tensor("out", [BLOC, C, T], F32, kind="ExternalOutput").ap()

    with tile.TileContext(nc) as tc:
        _emit(nc, tc, d_x, d_embT, d_qkvwT, d_embwT, d_projwT, d_gamma, d_beta,
              d_maskg, d_maskb, d_qkvb, d_embb, d_projb, d_out, with_proj_bias,
              with_qkv_bias)
    _split_excess_waits(nc)
    return nc


def _emit(nc, tc, d_x, d_embT, d_qkvwT, d_embwT, d_projwT, d_gamma, d_beta,
          d_maskg, d_maskb, d_qkvb, d_embb, d_projb, d_out, with_proj_bias,
          with_qkv_bias):
    from contextlib import ExitStack
    ctx = ExitStack()

    consts = ctx.enter_context(tc.tile_pool(name="consts", bufs=1))
    xt_pool = ctx.enter_context(tc.tile_pool(name="xt", bufs=2 * 4))
    h_pool = ctx.enter_context(tc.tile_pool(name="h", bufs=4))
    scratch = None  # sumsq scratch shares the h pool slots
    small = ctx.enter_context(tc.tile_pool(name="small", bufs=2))
    esb_pool = ctx.enter_context(tc.tile_pool(name="esbp", bufs=1))
    embw_pool = ctx.enter_context(tc.tile_pool(name="embw", bufs=2))
    qk_pool = ctx.enter_context(tc.tile_pool(name="qksb", bufs=16))
    vt_pool = ctx.enter_context(tc.tile_pool(name="vt", bufs=2))
    exp_pool = ctx.enter_context(tc.tile_pool(name="expS", bufs=18))
    rb_pool = ctx.enter_context(tc.tile_pool(name="rb", bufs=2))
    den_pool = ctx.enter_context(tc.tile_pool(name="den", bufs=1))
    a_pool = ctx.enter_context(tc.tile_pool(name="asb", bufs=2))
    o_pool = ctx.enter_context(tc.tile_pool(name="osb", bufs=2))
    dram_pool = ctx.enter_context(tc.tile_pool(name="drs", bufs=1, space="DRAM"))
    ps_gen = ctx.enter_context(tc.tile_pool(name="psgen", bufs=2, space="PSUM"))
    ps_pa = ctx.enter_context(tc.tile_pool(name="pspa", bufs=2, space="PSUM"))
    ps_bg = ctx.enter_context(tc.tile_pool(name="psbg", bufs=1, space="PSUM"))

    # ---------------- constants
    qkvw = consts.tile([128, 4, 3 * C], BF16)
    nc.sync.dma_start(
        out=qkvw[:], in_=d_qkvwT.rearrange("(t p) o -> p t o", p=128)
    )
    projw = consts.tile([128, 4, C], BF16)
    nc.sync.dma_start(out=projw[:], in_=d_projwT.rearrange("(t p) o -> p t o", p=128))
    gamma4 = consts.tile([128, 4], F32)
    nc.sync.dma_start(out=gamma4[:], in_=d_gamma[:])
    beta4 = consts.tile([128, 4], F32)
    nc.sync.dma_start(out=beta4[:], in_=d_beta[:])
    maskg = consts.tile([128, 8], F32)
    nc.sync.dma_start(out=maskg[:], in_=d_maskg[:])
    maskb = consts.tile([8, 128], F32)
    nc.sync.dma_start(out=maskb[:], in_=d_maskb[:])
    qkvb = consts.tile([128, 8], F32)
    nc.sync.dma_start(out=qkvb[:], in_=d_qkvb[:])
    embb2 = consts.tile([2, 2 * C], F32)
    nc.sync.dma_start(out=embb2[:], in_=d_embb[:])
    projb4 = None
    if with_proj_bias:
        projb4 = consts.tile([128, 4], F32)
        nc.sync.dma_start(out=projb4[:], in_=d_projb[:])
    eps8 = consts.tile([8, 1], F32)
    nc.vector.memset(eps8[:], EPS)

    # ---------------- x loads
    xt = {}
    for b in range(BLOC):
        for kt in range(4):
            t_ = xt_pool.tile([128, T], F32, tag="xt")
            nc.sync.dma_start(out=t_[:], in_=d_x[b, 128 * kt:128 * (kt + 1), :])
            xt[b, kt] = t_

    # ---------------- FiLM: silu(emb) @ emb_w.T for both batches at once
    embS = small.tile([128, 8, BLOC], F32, tag="embS")
    nc.sync.dma_start(out=embS[:], in_=d_embT.rearrange("(t p) b -> p t b", p=128))
    sEmb = small.tile([128, 8, BLOC], BF16, tag="sEmb")
    nc.scalar.activation(
        out=sEmb[:], in_=embS[:], func=mybir.ActivationFunctionType.Sigmoid
    )
    nc.vector.tensor_mul(sEmb[:], sEmb[:], embS[:])
    e_ps = ps_gen.tile([BLOC, 2 * C], F32, tag="psA")
    for kt in range(8):
        wtile = embw_pool.tile([128, 2 * C], BF16, tag="embw")
        nc.sync.dma_start(out=wtile[:], in_=d_embwT[128 * kt:128 * (kt + 1), :])
        for n in range(2):
            nc.tensor.matmul(
                e_ps[:, 512 * n:512 * (n + 1)],
                sEmb[:, kt, :],
                wtile[:, 512 * n:512 * (n + 1)],
                start=(kt == 0), stop=(kt == 7),
            )
    e_sb = esb_pool.tile([BLOC, 2 * C], F32, tag="esb")
    nc.vector.tensor_add(e_sb[:], e_ps[:], embb2[:])
    e_dram = dram_pool.tile([BLOC, 2 * C], F32, tag="edram")
    nc.sync.dma_start(out=e_dram[:], in_=e_sb[:])
    scsh = small.tile([128, 4, 2, BLOC], F32, tag="scsh")
    for b in range(BLOC):
        for sc in range(2):
            nc.sync.dma_start(
                out=scsh[:, :, sc, b],
                in_=e_dram[b, 512 * sc:512 * (sc + 1)].rearrange("(t p) -> p t", p=128),
            )

    # ---------------- group-norm stats (both batches)
    chan_ps = {}
    for b in range(BLOC):
        sq = small.tile([128, 8], F32, tag="sq")
        for kt in range(4):
            nc.vector.reduce_sum(
                out=sq[:, kt:kt + 1], in_=xt[b, kt][:], axis=mybir.AxisListType.X
            )
            scr = h_pool.tile([128, T], F32, tag="scr", bufs=1)
            nc.scalar.activation(
                out=scr[:], in_=xt[b, kt][:],
                func=mybir.ActivationFunctionType.Square,
                accum_out=sq[:, 4 + kt:5 + kt],
            )
        gsum = ps_gen.tile([8, 8], F32, tag="psA")
        nc.tensor.matmul(gsum[:], maskg[:], sq[:], start=True, stop=True)
        stats = small.tile([8, 8], F32, tag="stats")
        nc.vector.tensor_scalar_mul(stats[:, 0:4], gsum[:, 0:4], 1.0 / NELEM)
        msq = small.tile([8, 4], F32, tag="msq")
        nc.vector.tensor_scalar_mul(msq[:], gsum[:, 4:8], 1.0 / NELEM)
        mm2 = small.tile([8, 4], F32, tag="mm2")
        nc.vector.tensor_mul(mm2[:], stats[:, 0:4], stats[:, 0:4])
        var = small.tile([8, 4], F32, tag="var")
        nc.vector.tensor_sub(var[:], msq[:], mm2[:])
        sd = small.tile([8, 4], F32, tag="sd")
        nc.scalar.activation(
            out=sd[:], in_=var[:], func=mybir.ActivationFunctionType.Sqrt,
            bias=eps8[:], scale=1.0,
        )
        nc.vector.reciprocal(out=stats[:, 4:8], in_=sd[:])
        cp = ps_gen.tile([128, 8], F32, tag="psA")
        nc.tensor.matmul(cp[:], maskb[:], stats[:], start=True, stop=True)
        chan_ps[b] = cp

    # ---------------- per-channel affine A1, B1 and h = x*A1 + B1
    h = {}
    for b in range(BLOC):
        cp = chan_ps[b]
        a0 = small.tile([128, 4], F32, tag="a0")
        nc.vector.tensor_mul(a0[:], cp[:, 4:8], gamma4[:])
        t1 = small.tile([128, 4], F32, tag="t1")
        nc.vector.tensor_mul(t1[:], cp[:, 0:4], a0[:])
        b0 = small.tile([128, 4], F32, tag="b0")
        nc.vector.tensor_sub(b0[:], beta4[:], t1[:])
        t2 = small.tile([128, 4], F32, tag="t2")
        nc.vector.tensor_mul(t2[:], a0[:], scsh[:, :, 0, b])
        a1 = small.tile([128, 4], F32, tag="a1")
        nc.vector.tensor_add(a1[:], a0[:], t2[:])
        t3 = small.tile([128, 4], F32, tag="t3")
        nc.vector.tensor_mul(t3[:], b0[:], scsh[:, :, 0, b])
        t4 = small.tile([128, 4], F32, tag="t4")
        nc.vector.tensor_add(t4[:], b0[:], t3[:])
        b1 = small.tile([128, 4], F32, tag="b1")
        nc.vector.tensor_add(b1[:], t4[:], scsh[:, :, 1, b])
        for kt in range(4):
            ht = h_pool.tile([128, T], BF16, tag="h")
            nc.vector.tensor_scalar(
                out=ht[:], in0=xt[b, kt][:],
                scalar1=a1[:, kt:kt + 1], scalar2=b1[:, kt:kt + 1],
                op0=mybir.AluOpType.mult, op1=mybir.AluOpType.add,
            )
            h[b, kt] = ht

    # ---------------- QKV: q,k head-pair tiles + transposed V with ones column
    q_sb, k_sb, vt_sb = {}, {}, {}

    def qkv_tasks(b, pool):
        """Yield one closure per PSUM-group of batch-b QKV work."""
        def qk_group(grp, store, bcol, hp):
            def run():
                ps = pool.tile([128, T], F32, tag="psA" if pool is ps_gen else "psBG")
                for n in range(2):
                    for kt in range(4):
                        nc.tensor.matmul(
                            ps[:, 512 * n:512 * (n + 1)],
                            qkvw[:, kt, 512 * grp + 128 * hp:512 * grp + 128 * (hp + 1)],
                            h[b, kt][:, 512 * n:512 * (n + 1)],
                            start=(kt == 0), stop=(kt == 3),
                        )
                sb = qk_pool.tile([128, T], BF16, tag="qksb")
                if with_qkv_bias:
                    nc.vector.tensor_scalar(
                        out=sb[:], in0=ps[:],
                        scalar1=qkvb[:, bcol + hp:bcol + hp + 1],
                        scalar2=None, op0=mybir.AluOpType.add,
                    )
                else:
                    nc.vector.tensor_copy(out=sb[:], in_=ps[:])
                store[b, hp] = sb
            return run

        def vt_group(st):
            def run():
                vt = vt_sb[b]
                ps = pool.tile([128, C], F32, tag="psA" if pool is ps_gen else "psBG")
                for kt in range(4):
                    nc.tensor.matmul(
                        ps[:, :],
                        h[b, kt][:, 128 * st:128 * (st + 1)],
                        qkvw[:, kt, 1024:1536],
                        start=(kt == 0), stop=(kt == 3),
                    )
                nc.vector.tensor_copy(
                    out=vt[:, st, :, 0:CH],
                    in_=ps[:].rearrange("p (h c) -> p h c", c=CH),
                )
            return run

        def vt_init():
            vt = vt_pool.tile([128, 8, NH, CH + 1], BF16, tag="vt")
            nc.vector.memset(vt[:, :, :, CH:CH + 1], 1.0)
            vt_sb[b] = vt

    # interleave q/k pairs so attention head-pair hp can start after 2 evacs
        tasks = []
        for hp in range(4):
            tasks.append(qk_group(0, q_sb, 0, hp))
            tasks.append(qk_group(1, k_sb, 4, hp))
        tasks.append(vt_init)
        for st in range(8):
            tasks.append(vt_group(st))
        return tasks

    # batch 0 QKV runs upfront; the first half of batch 1's QKV fills the
    # latency-bound startup window, the rest interleaves into b0's attention
    for t in qkv_tasks(0, ps_gen):
        t()
    b1_tasks = qkv_tasks(1, ps_gen)
    for t in b1_tasks[:9]:
        t()
    b1_rest = b1_tasks[9:]

    # ---------------- attention per batch with background-task interleaving
    a_sb_by_batch = {}

    def proj_tasks(b):
        def proj_group(m4):
            def run():
                a_sb = a_sb_by_batch[b]
                ps = ps_bg.tile([128, T], F32, tag="psBG")
                for n in range(2):
                    for kt in range(4):
                        nc.tensor.matmul(
                            ps[:, 512 * n:512 * (n + 1)],
                            projw[:, kt, 128 * m4:128 * (m4 + 1)],
                            a_sb[:, kt, 512 * n:512 * (n + 1)],
                            start=(kt == 0), stop=(kt == 3),
                        )
                o = o_pool.tile([128, T], F32, tag="osb")
                nc.vector.tensor_add(o[:], ps[:], xt[b, m4][:])
                if projb4 is not None:
                    nc.vector.tensor_scalar(
                        out=o[:], in0=o[:], scalar1=projb4[:, m4:m4 + 1],
                        scalar2=None, op0=mybir.AluOpType.add,
                    )
                nc.sync.dma_start(out=d_out[b, 128 * m4:128 * (m4 + 1), :], in_=o[:])
            return run
        return [proj_group(m4) for m4 in range(4)]

    def run_attention(b, bg_tasks, bg_every):
        a_sb = a_pool.tile([128, 4, T], BF16, tag="asb")
        # denominators packed at partition offsets {0,32,64,96}; one tile per
        # half (heads 0-3 / heads 4-7) so the first half normalizes early
        den_lo = den_pool.tile([97, T], F32, tag="den")
        nc.vector.memset(den_lo[:], 1.0)
        den_hi = den_pool.tile([97, T], F32, tag="den")
        nc.vector.memset(den_hi[:], 1.0)
        den_halves = (den_lo, den_hi)
        vt = vt_sb[b]
        bg = list(bg_tasks)
        bg_i = 0
        slot = 0

        pa = {}    # (head, n) -> psum [CH+1, 512]
        ex = {}    # (head, st) -> exp tile

        def emit_av_step(hh, step):
            n, st = divmod(step, 8) if step < 8 else (1, step - 8)
            n = 0 if step < 8 else 1
            st = step if step < 8 else step - 8
            if st == 0:
                pa_t = ps_pa.tile([CH + 1, 512], F32, tag="pa")
                pa[hh, n] = pa_t
            nc.tensor.matmul(
                pa[hh, n][:, :],
                vt[:, st, hh, :],
                ex[hh, st][:, 512 * n:512 * (n + 1)],
                start=(st == 0), stop=(st == 7),
            )
            if st == 7:
                pslot = 32 * (hh % 4)
                nc.vector.tensor_copy(
                    out=a_sb[64 * (hh % 2):64 * (hh % 2) + 64, hh // 2,
                             512 * n:512 * (n + 1)],
                    in_=pa[hh, n][0:CH, :],
                )
                nc.vector.tensor_copy(
                    out=den_halves[hh // 4][pslot:pslot + 1, 512 * n:512 * (n + 1)],
                    in_=pa[hh, n][CH:CH + 1, :],
                )
                del pa[hh, n]
                if n == 1:
                    for s2 in range(8):
                        del ex[hh, s2]

        den_dram = dram_pool.tile([NH, T], F32, tag="den_dram", bufs=2)

        def normalize_half(half):
            # heads 4h..4h+3 -> one reciprocal, broadcast, two in-place mults
            dn = den_halves[half]
            nc.vector.reciprocal(out=dn[:], in_=dn[:])
            for j in range(4):
                hh2 = 4 * half + j
                nc.sync.dma_start(
                    out=den_dram[hh2, :], in_=dn[32 * j:32 * j + 1, :]
                )
            for ct in (2 * half, 2 * half + 1):
                rb = rb_pool.tile([128, T], F32, tag="rb")
                for par2 in range(2):
                    rd_ap = den_dram[2 * ct + par2, :]
                    nc.sync.dma_start(
                        out=rb[64 * par2:64 * par2 + 64, :],
                        in_=bass.AP(tensor=rd_ap.tensor, offset=rd_ap.offset,
                                    ap=[[0, CH]] + [list(d) for d in rd_ap.ap]),
                    )
                sl = a_sb[:, ct, :]
                nc.vector.tensor_mul(sl, sl, rb[:])

        prev = None
        for hh in range(NH):
            hp, par = hh // 2, 64 * (hh % 2)
            for st in range(8):
                qk = ps_gen.tile([128, T], F32, tag="psA")
                for n in range(2):
                    nc.tensor.matmul(
                        qk[:, 512 * n:512 * (n + 1)],
                        k_sb[b, hp][par:par + 64, 128 * st:128 * (st + 1)],
                        q_sb[b, hp][par:par + 64, 512 * n:512 * (n + 1)],
                        start=True, stop=True,
                    )
                et = exp_pool.tile([128, T], BF16, tag="expS")
                nc.scalar.activation(
                    out=et[:], in_=qk[:], func=mybir.ActivationFunctionType.Exp
                )
                ex[hh, st] = et
                if prev is not None:
                    emit_av_step(prev, 2 * st)
                    emit_av_step(prev, 2 * st + 1)
                slot += 1
                if bg_i < len(bg) and slot % bg_every == 0:
                    bg[bg_i]()
                    bg_i += 1
            prev = hh
            if hh == 5:
                normalize_half(0)
        for step in range(16):
            emit_av_step(prev, step)
        while bg_i < len(bg):
            bg[bg_i]()
            bg_i += 1
        normalize_half(1)
        a_sb_by_batch[b] = a_sb

    run_attention(0, b1_rest, bg_every=7)
    run_attention(1, proj_tasks(0), bg_every=12)
    for t in proj_tasks(1):
        t()

    ctx.close()


# ------------------------------------------------------------------- host side
def _host_prep(x, emb, gn_gamma, gn_beta, emb_w, emb_b, qkv_w, qkv_b,
               proj_w, proj_b):
    x = np.ascontiguousarray(np.asarray(x, np.float32).reshape(B, C, T))
    embT = np.ascontiguousarray(np.asarray(emb, np.float32).T)           # [E, B]

    qkv_wT = np.array(np.asarray(qkv_w, np.float32).T)                   # [C, 3C]
    wview = qkv_wT.reshape(C, NH, 3, CH)
    wview[:, :, 0, :] *= SCALE
    wview[:, :, 1, :] *= SCALE
    # permute cout: [q head 0..7 | k head 0..7 | v head 0..7] so pair/group
    # slices are contiguous for the PE stationary operand
    o = np.arange(3 * C).reshape(NH, 3, CH)
    perm = np.concatenate([o[:, 0, :].ravel(), o[:, 1, :].ravel(), o[:, 2, :].ravel()])
    qkv_wT_bf = np.ascontiguousarray(qkv_wT[:, perm].astype(ml_dtypes.bfloat16))

    emb_wT = np.ascontiguousarray(np.asarray(emb_w, np.float32).T.astype(ml_dtypes.bfloat16))  # [E, 2C]
    proj_wT = np.ascontiguousarray(np.asarray(proj_w, np.float32).T.astype(ml_dtypes.bfloat16))  # [C, C]

    gamma4 = np.ascontiguousarray(np.asarray(gn_gamma, np.float32).reshape(4, 128).T)
    beta4 = np.ascontiguousarray(np.asarray(gn_beta, np.float32).reshape(4, 128).T)

    p = np.arange(128)
    g = np.arange(8)
    mask_g = (p[:, None] // GSIZE == g[None, :]).astype(np.float32)      # [128, 8]
    mask_b = np.ascontiguousarray(mask_g.T)                              # [8, 128]

    qkv_b = np.asarray(qkv_b, np.float32)
    qb = qkv_b.reshape(NH, 3, CH)
    qkvb_qk = np.zeros((128, 8), np.float32)
    for hp in range(4):
        for par in range(2):
            qkvb_qk[64 * par:64 * par + 64, hp] = qb[2 * hp + par, 0] * SCALE
            qkvb_qk[64 * par:64 * par + 64, 4 + hp] = qb[2 * hp + par, 1] * SCALE

    emb_b2 = np.ascontiguousarray(
        np.broadcast_to(np.asarray(emb_b, np.float32)[None, :], (2, 2 * C))
    )

    # effective proj bias: proj_b + proj_w @ v_bias (v bias commutes with softmax)
    pb_eff = (np.asarray(proj_b, np.float64)
              + np.asarray(proj_w, np.float64) @ qb[:, 2, :].reshape(C).astype(np.float64))
    pb_eff = pb_eff.astype(np.float32)
    with_proj_bias = bool(np.any(pb_eff != 0))
    proj_b4 = np.ascontiguousarray(pb_eff.reshape(4, 128).T)

    shared = dict(qkv_wT=qkv_wT_bf, emb_wT=emb_wT, proj_wT=proj_wT,
                  gamma4=gamma4, beta4=beta4, mask_g=mask_g, mask_b=mask_b,
                  qkvb_qk=qkvb_qk, emb_b2=emb_b2)
    if with_proj_bias:
        shared["proj_b4"] = proj_b4

    with_qkv_bias = bool(np.any(qkvb_qk))
    in_maps = []
    for i in range(NCORES):
        m = dict(shared)
        m["x"] = np.ascontiguousarray(x[BLOC * i:BLOC * (i + 1)])
        m["embT"] = np.ascontiguousarray(embT[:, BLOC * i:BLOC * (i + 1)])
        in_maps.append(m)
    return in_maps, with_proj_bias, with_qkv_bias


_PROGRAM_CACHE = {}


def kernel(**inputs):
    global LAST_EXEC_NS, LAST_TRACE_DIR
    in_maps, with_proj_bias, with_qkv_bias = _host_prep(**inputs)
    key = (with_proj_bias, with_qkv_bias)
    if key not in _PROGRAM_CACHE:
        _PROGRAM_CACHE[key] = build_program(with_proj_bias, with_qkv_bias)
    nc = _PROGRAM_CACHE[key]

    kwargs = {}
    if TRACE:
        import tempfile
        _install_prof_shim()
        kwargs = dict(trace=True, tmpdir=tempfile.mkdtemp(prefix="attn_trace_"))
    res = run_bass_kernel_spmd(nc, in_maps, core_ids=list(range(NCORES)), **kwargs)
    LAST_EXEC_NS = res.exec_time_ns
    LAST_TRACE_DIR = kwargs.get("tmpdir")

    out = np.empty((B, C, T), np.float32)
    for i in range(NCORES):
        out[BLOC * i:BLOC * (i + 1)] = res.results[i]["out"]
    return out.reshape(B, C, HSP, WSP)


def _install_prof_shim():
    """Register the NTFF profile hook (missing antenv.axon_hooks in this image)."""
    import sys
    import types
    try:
        import antenv
        if not hasattr(antenv, "axon_hooks"):
            mod = types.ModuleType("antenv.axon_hooks")
            mod._hook = None
            mod.set_axon_ntff_profile_hook = lambda h: setattr(mod, "_hook", h)
            mod.get_axon_ntff_profile_hook = lambda: mod._hook
            sys.modules["antenv.axon_hooks"] = mod
            antenv.axon_hooks = mod
        from antenv.axon_hooks import (
            get_axon_ntff_profile_hook,
            set_axon_ntff_profile_hook,
        )
        if get_axon_ntff_profile_hook() is None:
            from trn_agent_boot.trn_boot import _ntff_profile_via_ctypes
            set_axon_ntff_profile_hook(
                _ntff_profile_via_ctypes("/opt/axon/libaxon_pjrt.so")
            )
        from concourse import bass_utils
        bass_utils.upload_artifacts = lambda tmpdir: "local://" + tmpdir
    except Exception as exc:  # profiling is best-effort
        print("prof shim install failed:", exc)


# revision 38
# speedup vs baseline: 1.2494x; 1.2494x over previous
"""Trainium2 Bass kernel for nn_AttentionBlock (GroupNorm + FiLM + MHA + proj + residual).

Sharding: data-parallel over batch B=16 across 8 NeuronCores (2 batches/core).
All inputs are taken in full, sharded/laid-out on host, run via
run_bass_kernel_spmd, and the full [16, 512, 32, 32] output is reassembled.

Self-contained: only imports the environment toolchain (concourse/bass_rust).
"""

import os

import numpy as np
import ml_dtypes

import bass_rust
import concourse.bass as bass
import concourse.tile as tile
from concourse import mybir
from concourse.bass_utils import run_bass_kernel_spmd

# ---------------------------------------------------------------- problem dims
B, C, HSP, WSP = 16, 512, 32, 32
T = HSP * WSP            # 1024
E = 1024
NCORES = 8
BLOC = B // NCORES       # 2 batches per core
NG = 32                  # groups
NH = 8                   # heads
CH = C // NH             # 64 head channels
EPS = 1e-5
GSIZE = C // NG          # 16 channels per group
NELEM = GSIZE * T        # elements per group (16384)
SCALE = 1.0 / np.sqrt(np.sqrt(float(CH)))

F32 = mybir.dt.float32
F32R = mybir.dt.float32r
BF16 = mybir.dt.bfloat16

# set by test.py to enable NTFF tracing; harness leaves it off
TRACE = bool(int(os.environ.get("BASS_ATTN_TRACE", "0")))
LAST_EXEC_NS = None
LAST_TRACE_DIR = None


# ------------------------------------------------- sync-wait splitting post-pass
def _split_excess_waits(nc, max_waits=2):
    """walrus in this container encodes at most 2 sem waits per instruction.

    Move excess waits onto freshly inserted NOPs on the same engine right
    before the offending instruction (identical semantics: same point in that
    engine's program order).
    """
    n_id = 0
    for f in nc.m.functions:
        for blk in f.blocks:
            il = blk.instructions  # live list
            idx = 0
            while idx < len(il):
                inst = il[idx]
                si = inst.sync_info
                if si is None or not si.on_wait:
                    idx += 1
                    continue
                budget = 1
                if len(si.on_wait) > budget:
                    waits = list(si.on_wait)
                    keep = waits[-budget:]
                    excess = waits[:-budget]
                    pos = idx
                    for j in range(0, len(excess), 1):
                        grp = excess[j:j + 1]
                        nop = mybir.InstNoOp(name=f"wsplit-{n_id}", ins=[], outs=[])
                        n_id += 1
                        nop.engine = inst.engine
                        nop.sync_info = mybir.SyncInfo(on_wait=grp, on_update=[])
                        try:
                            nc.register_instruction(nop, overwrite=True)
                        except Exception:
                            pass
                        il.insert(pos, nop)
                        pos += 1
                        idx += 1
                    inst.sync_info = mybir.SyncInfo(
                        on_wait=keep, on_update=list(si.on_update or [])
                    )
                idx += 1


def _install_tile_tail_patch():
    return


# ------------------------------------------------------------- device program
def build_program(with_proj_bias: bool, with_qkv_bias: bool = False):
    _install_tile_tail_patch()
    nc = bass.Bass("TRN2", target_bir_lowering=False, debug=False)

    d_x = nc.dram_tensor("x", [BLOC, C, T], F32, kind="ExternalInput").ap()
    d_embT = nc.dram_tensor("embT", [E, BLOC], F32, kind="ExternalInput").ap()
    d_qkvwT = nc.dram_tensor("qkv_wT", [C, 3 * C], BF16, kind="ExternalInput").ap()
    d_embwT = nc.dram_tensor("emb_wT", [E, 2 * C], BF16, kind="ExternalInput").ap()
    d_projwT = nc.dram_tensor("proj_wT", [C, C], BF16, kind="ExternalInput").ap()
    d_gamma = nc.dram_tensor("gamma4", [128, 4], F32, kind="ExternalInput").ap()
    d_beta = nc.dram_tensor("beta4", [128, 4], F32, kind="ExternalInput").ap()
    d_maskg = nc.dram_tensor("mask_g", [128, 8], F32, kind="ExternalInput").ap()
    d_maskb = nc.dram_tensor("mask_b", [8, 128], F32, kind="ExternalInput").ap()
    d_qkvb = nc.dram_tensor("qkvb_qk", [128, 8], F32, kind="ExternalInput").ap()
    d_embb = nc.dram_tensor("emb_b2", [2, 2 * C], F32, kind="ExternalInput").ap()
    d_projb = (nc.dram_tensor("proj_b4", [128, 4], F32, kind="ExternalInput").ap()
               if with_proj_bias else None)
    d_out = nc.dram_tensor("out", [BLOC, C, T], F32, kind="ExternalOutput").ap()

    with tile.TileContext(nc) as tc:
        _emit(nc, tc, d_x, d_embT, d_qkvwT, d_embwT, d_projwT, d_gamma, d_beta,
              d_maskg, d_maskb, d_qkvb, d_embb, d_projb, d_out, with_proj_bias,
              with_qkv_bias)
    _split_excess_waits(nc)
    return nc


def _emit(nc, tc, d_x, d_embT, d_qkvwT, d_embwT, d_projwT, d_gamma, d_beta,
          d_maskg, d_maskb, d_qkvb, d_embb, d_projb, d_out, with_proj_bias,
          with_qkv_bias):
    from contextlib import ExitStack
    ctx = ExitStack()

    consts = ctx.enter_context(tc.tile_pool(name="consts", bufs=1))
    xt_pool = ctx.enter_context(tc.tile_pool(name="xt", bufs=2 * 4))
    h_pool = ctx.enter_context(tc.tile_pool(name="h", bufs=4))
    scratch = None  # sumsq scratch shares the h pool slots
    small = ctx.enter_context(tc.tile_pool(name="small", bufs=2))
    esb_pool = ctx.enter_context(tc.tile_pool(name="esbp", bufs=1))
    embw_pool = ctx.enter_context(tc.tile_pool(name="embw", bufs=2))
    qk_pool = ctx.enter_context(tc.tile_pool(name="qksb", bufs=16))
    vt_pool = ctx.enter_context(tc.tile_pool(name="vt", bufs=2))
    exp_pool = ctx.enter_context(tc.tile_pool(name="expS", bufs=18))
    rb_pool = ctx.enter_context(tc.tile_pool(name="rb", bufs=2))
    den_pool = ctx.enter_context(tc.tile_pool(name="den", bufs=1))
    a_pool = ctx.enter_context(tc.tile_pool(name="asb", bufs=2))
    o_pool = ctx.enter_context(tc.tile_pool(name="osb", bufs=2))
    dram_pool = ctx.enter_context(tc.tile_pool(name="drs", bufs=1, space="DRAM"))
    ps_gen = ctx.enter_context(tc.tile_pool(name="psgen", bufs=2, space="PSUM"))
    ps_pa = ctx.enter_context(tc.tile_pool(name="pspa", bufs=2, space="PSUM"))
    ps_bg = ctx.enter_context(tc.tile_pool(name="psbg", bufs=1, space="PSUM"))

    # ---------------- constants
    qkvw = consts.tile([128, 4, 3 * C], BF16)
    nc.sync.dma_start(
        out=qkvw[:], in_=d_qkvwT.rearrange("(t p) o -> p t o", p=128)
    )
    projw = consts.tile([128, 4, C], BF16)
    nc.sync.dma_start(out=projw[:], in_=d_projwT.rearrange("(t p) o -> p t o", p=128))
    gamma4 = consts.tile([128, 4], F32)
    nc.sync.dma_start(out=gamma4[:], in_=d_gamma[:])
    beta4 = consts.tile([128, 4], F32)
    nc.sync.dma_start(out=beta4[:], in_=d_beta[:])
    maskg = consts.tile([128, 8], F32)
    nc.sync.dma_start(out=maskg[:], in_=d_maskg[:])
    maskb = consts.tile([8, 128], F32)
    nc.sync.dma_start(out=maskb[:], in_=d_maskb[:])
    qkvb = consts.tile([128, 8], F32)
    nc.sync.dma_start(out=qkvb[:], in_=d_qkvb[:])
    embb2 = consts.tile([2, 2 * C], F32)
    nc.sync.dma_start(out=embb2[:], in_=d_embb[:])
    projb4 = None
    if with_proj_bias:
        projb4 = consts.tile([128, 4], F32)
        nc.sync.dma_start(out=projb4[:], in_=d_projb[:])
    eps8 = consts.tile([8, 1], F32)
    nc.vector.memset(eps8[:], EPS)

    # ---------------- x loads
    xt = {}
    for b in range(BLOC):
        for kt in range(4):
            t_ = xt_pool.tile([128, T], F32, tag="xt")
            nc.sync.dma_start(out=t_[:], in_=d_x[b, 128 * kt:128 * (kt + 1), :])
            xt[b, kt] = t_

    # ---------------- FiLM: silu(emb) @ emb_w.T for both batches at once
    embS = small.tile([128, 8, BLOC], F32, tag="embS")
    nc.sync.dma_start(out=embS[:], in_=d_embT.rearrange("(t p) b -> p t b", p=128))
    sEmb = small.tile([128, 8, BLOC], BF16, tag="sEmb")
    nc.scalar.activation(
        out=sEmb[:], in_=embS[:], func=mybir.ActivationFunctionType.Sigmoid
    )
    nc.vector.tensor_mul(sEmb[:], sEmb[:], embS[:])
    e_ps = ps_gen.tile([BLOC, 2 * C], F32, tag="psA")
    for kt in range(8):
        wtile = embw_pool.tile([128, 2 * C], BF16, tag="embw")
        nc.sync.dma_start(out=wtile[:], in_=d_embwT[128 * kt:128 * (kt + 1), :])
        for n in range(2):
            nc.tensor.matmul(
                e_ps[:, 512 * n:512 * (n + 1)],
                sEmb[:, kt, :],
                wtile[:, 512 * n:512 * (n + 1)],
                start=(kt == 0), stop=(kt == 7),
            )
    e_sb = esb_pool.tile([BLOC, 2 * C], F32, tag="esb")
    nc.vector.tensor_add(e_sb[:], e_ps[:], embb2[:])
    e_dram = dram_pool.tile([BLOC, 2 * C], F32, tag="edram")
    nc.sync.dma_start(out=e_dram[:], in_=e_sb[:])
    scsh = small.tile([128, 4, 2, BLOC], F32, tag="scsh")
    for b in range(BLOC):
        for sc in range(2):
            nc.sync.dma_start(
                out=scsh[:, :, sc, b],
                in_=e_dram[b, 512 * sc:512 * (sc + 1)].rearrange("(t p) -> p t", p=128),
            )

    # ---------------- group-norm stats (both batches)
    chan_ps = {}
    for b in range(BLOC):
        sq = small.tile([128, 8], F32, tag="sq")
        for kt in range(4):
            nc.vector.reduce_sum(
                out=sq[:, kt:kt + 1], in_=xt[b, kt][:], axis=mybir.AxisListType.X
            )
            scr = h_pool.tile([128, T], F32, tag="scr", bufs=1)
            nc.scalar.activation(
                out=scr[:], in_=xt[b, kt][:],
                func=mybir.ActivationFunctionType.Square,
                accum_out=sq[:, 4 + kt:5 + kt],
            )
        gsum = ps_gen.tile([8, 8], F32, tag="psA")
        nc.tensor.matmul(gsum[:], maskg[:], sq[:], start=True, stop=True)
        stats = small.tile([8, 8], F32, tag="stats")
        nc.vector.tensor_scalar_mul(stats[:, 0:4], gsum[:, 0:4], 1.0 / NELEM)
        msq = small.tile([8, 4], F32, tag="msq")
        nc.vector.tensor_scalar_mul(msq[:], gsum[:, 4:8], 1.0 / NELEM)
        mm2 = small.tile([8, 4], F32, tag="mm2")
        nc.vector.tensor_mul(mm2[:], stats[:, 0:4], stats[:, 0:4])
        var = small.tile([8, 4], F32, tag="var")
        nc.vector.tensor_sub(var[:], msq[:], mm2[:])
        sd = small.tile([8, 4], F32, tag="sd")
        nc.scalar.activation(
            out=sd[:], in_=var[:], func=mybir.ActivationFunctionType.Sqrt,
            bias=eps8[:], scale=1.0,
        )
        nc.vector.reciprocal(out=stats[:, 4:8], in_=sd[:])
        cp = ps_gen.tile([128, 8], F32, tag="psA")
        nc.tensor.matmul(cp[:], maskb[:], stats[:], start=True, stop=True)
        chan_ps[b] = cp

    # ---------------- per-channel affine A1, B1 and h = x*A1 + B1
    h = {}
    for b in range(BLOC):
        cp = chan_ps[b]
        a0 = small.tile([128, 4], F32, tag="a0")
        nc.vector.tensor_mul(a0[:], cp[:, 4:8], gamma4[:])
        t1 = small.tile([128, 4], F32, tag="t1")
        nc.vector.tensor_mul(t1[:], cp[:, 0:4], a0[:])
        b0 = small.tile([128, 4], F32, tag="b0")
        nc.vector.tensor_sub(b0[:], beta4[:], t1[:])
        t2 = small.tile([128, 4], F32, tag="t2")
        nc.vector.tensor_mul(t2[:], a0[:], scsh[:, :, 0, b])
        a1 = small.tile([128, 4], F32, tag="a1")
        nc.vector.tensor_add(a1[:], a0[:], t2[:])
        t3 = small.tile([128, 4], F32, tag="t3")
        nc.vector.tensor_mul(t3[:], b0[:], scsh[:, :, 0, b])
        t4 = small.tile([128, 4], F32, tag="t4")
        nc.vector.tensor_add(t4[:], b0[:], t3[:])
        b1 = small.tile([128, 4], F32, tag="b1")
        nc.vector.tensor_add(b1[:], t4[:], scsh[:, :, 1, b])
        for kt in range(4):
            ht = h_pool.tile([128, T], BF16, tag="h")
            nc.vector.tensor_scalar(
                out=ht[:], in0=xt[b, kt][:],
                scalar1=a1[:, kt:kt + 1], scalar2=b1[:, kt:kt + 1],
                op0=mybir.AluOpType.mult, op1=mybir.AluOpType.add,
            )
            h[b, kt] = ht

    # ---------------- QKV: q,k head-pair tiles + transposed V with ones column
    q_sb, k_sb, vt_sb = {}, {}, {}

    def qkv_tasks(b, pool):
        """Yield one closure per PSUM-group of batch-b QKV work."""
        def qk_group(grp, store, bcol, hp):
            def run():
                ps = pool.tile([128, T], F32, tag="psA" if pool is ps_gen else "psBG")
                for n in range(2):
                    for kt in range(4):
                        nc.tensor.matmul(
                            ps[:, 512 * n:512 * (n + 1)],
                            qkvw[:, kt, 512 * grp + 128 * hp:512 * grp + 128 * (hp + 1)],
                            h[b, kt][:, 512 * n:512 * (n + 1)],
                            start=(kt == 0), stop=(kt == 3),
                        )
                sb = qk_pool.tile([128, T], BF16, tag="qksb")
                if with_qkv_bias:
                    nc.vector.tensor_scalar(
                        out=sb[:], in0=ps[:],
                        scalar1=qkvb[:, bcol + hp:bcol + hp + 1],
                        scalar2=None, op0=mybir.AluOpType.add,
                    )
                else:
                    nc.vector.tensor_copy(out=sb[:], in_=ps[:])
                store[b, hp] = sb
            return run

        def vt_group(st):
            def run():
                vt = vt_sb[b]
                ps = pool.tile([128, C], F32, tag="psA" if pool is ps_gen else "psBG")
                for kt in range(4):
                    nc.tensor.matmul(
                        ps[:, :],
                        h[b, kt][:, 128 * st:128 * (st + 1)],
                        qkvw[:, kt, 1024:1536],
                        start=(kt == 0), stop=(kt == 3),
                    )
                nc.vector.tensor_copy(
                    out=vt[:, st, :, 0:CH],
                    in_=ps[:].rearrange("p (h c) -> p h c", c=CH),
                )
            return run

        def vt_init():
            vt = vt_pool.tile([128, 8, NH, CH + 1], BF16, tag="vt")
            nc.vector.memset(vt[:, :, :, CH:CH + 1], 1.0)
            vt_sb[b] = vt

    # interleave q/k pairs so attention head-pair hp can start after 2 evacs
        tasks = []
        for hp in range(4):
            tasks.append(qk_group(0, q_sb, 0, hp))
            tasks.append(qk_group(1, k_sb, 4, hp))
        tasks.append(vt_init)
        for st in range(8):
            tasks.append(vt_group(st))
        return tasks

    # batch 0 QKV runs upfront; batch 1 is interleaved into b0's attention
    for t in qkv_tasks(0, ps_gen):
        t()

    # ---------------- attention per batch with background-task interleaving
    a_sb_by_batch = {}

    def proj_tasks(b):
        def proj_group(m4):
            def run():
                a_sb = a_sb_by_batch[b]
                ps = ps_bg.tile([128, T], F32, tag="psBG")
                for n in range(2):
                    for kt in range(4):
                        nc.tensor.matmul(
                            ps[:, 512 * n:512 * (n + 1)],
                            projw[:, kt, 128 * m4:128 * (m4 + 1)],
                            a_sb[:, kt, 512 * n:512 * (n + 1)],
                            start=(kt == 0), stop=(kt == 3),
                        )
                o = o_pool.tile([128, T], F32, tag="osb")
                nc.vector.tensor_add(o[:], ps[:], xt[b, m4][:])
                if projb4 is not None:
                    nc.vector.tensor_scalar(
                        out=o[:], in0=o[:], scalar1=projb4[:, m4:m4 + 1],
                        scalar2=None, op0=mybir.AluOpType.add,
                    )
                nc.sync.dma_start(out=d_out[b, 128 * m4:128 * (m4 + 1), :], in_=o[:])
            return run
        return [proj_group(m4) for m4 in range(4)]

    def run_attention(b, bg_tasks, bg_every):
        a_sb = a_pool.tile([128, 4, T], BF16, tag="asb")
        # denominators packed at partition offsets {0,32,64,96}; one tile per
        # half (heads 0-3 / heads 4-7) so the first half normalizes early
        den_lo = den_pool.tile([97, T], F32, tag="den")
        nc.vector.memset(den_lo[:], 1.0)
        den_hi = den_pool.tile([97, T], F32, tag="den")
        nc.vector.memset(den_hi[:], 1.0)
        den_halves = (den_lo, den_hi)
        vt = vt_sb[b]
        bg = list(bg_tasks)
        bg_i = 0
        slot = 0

        pa = {}    # (head, n) -> psum [CH+1, 512]
        ex = {}    # (head, st) -> exp tile

        def emit_av_step(hh, step):
            n, st = divmod(step, 8) if step < 8 else (1, step - 8)
            n = 0 if step < 8 else 1
            st = step if step < 8 else step - 8
            if st == 0:
                pa_t = ps_pa.tile([CH + 1, 512], F32, tag="pa")
                pa[hh, n] = pa_t
            nc.tensor.matmul(
                pa[hh, n][:, :],
                vt[:, st, hh, :],
                ex[hh, st][:, 512 * n:512 * (n + 1)],
                start=(st == 0), stop=(st == 7),
            )
            if st == 7:
                pslot = 32 * (hh % 4)
                nc.vector.tensor_copy(
                    out=a_sb[64 * (hh % 2):64 * (hh % 2) + 64, hh // 2,
                             512 * n:512 * (n + 1)],
                    in_=pa[hh, n][0:CH, :],
                )
                nc.vector.tensor_copy(
                    out=den_halves[hh // 4][pslot:pslot + 1, 512 * n:512 * (n + 1)],
                    in_=pa[hh, n][CH:CH + 1, :],
                )
                del pa[hh, n]
                if n == 1:
                    for s2 in range(8):
                        del ex[hh, s2]

        den_dram = dram_pool.tile([NH, T], F32, tag="den_dram", bufs=2)

        def normalize_half(half):
            # heads 4h..4h+3 -> one reciprocal, broadcast, two in-place mults
            dn = den_halves[half]
            nc.vector.reciprocal(out=dn[:], in_=dn[:])
            for j in range(4):
                hh2 = 4 * half + j
                nc.sync.dma_start(
                    out=den_dram[hh2, :], in_=dn[32 * j:32 * j + 1, :]
                )
            for ct in (2 * half, 2 * half + 1):
                rb = rb_pool.tile([128, T], F32, tag="rb")
                for par2 in range(2):
                    rd_ap = den_dram[2 * ct + par2, :]
                    nc.sync.dma_start(
                        out=rb[64 * par2:64 * par2 + 64, :],
                        in_=bass.AP(tensor=rd_ap.tensor, offset=rd_ap.offset,
                                    ap=[[0, CH]] + [list(d) for d in rd_ap.ap]),
                    )
                sl = a_sb[:, ct, :]
                nc.vector.tensor_mul(sl, sl, rb[:])

        prev = None
        for hh in range(NH):
            hp, par = hh // 2, 64 * (hh % 2)
            for st in range(8):
                qk = ps_gen.tile([128, T], F32, tag="psA")
                for n in range(2):
                    nc.tensor.matmul(
                        qk[:, 512 * n:512 * (n + 1)],
                        k_sb[b, hp][par:par + 64, 128 * st:128 * (st + 1)],
                        q_sb[b, hp][par:par + 64, 512 * n:512 * (n + 1)],
                        start=True, stop=True,
                    )
                et = exp_pool.tile([128, T], BF16, tag="expS")
                nc.scalar.activation(
                    out=et[:], in_=qk[:], func=mybir.ActivationFunctionType.Exp
                )
                ex[hh, st] = et
                if prev is not None:
                    emit_av_step(prev, 2 * st)
                    emit_av_step(prev, 2 * st + 1)
                slot += 1
                if bg_i < len(bg) and slot % bg_every == 0:
                    bg[bg_i]()
                    bg_i += 1
            prev = hh
            if hh == 5:
                normalize_half(0)
        for step in range(16):
            emit_av_step(prev, step)
        while bg_i < len(bg):
            bg[bg_i]()
            bg_i += 1
        normalize_half(1)
        a_sb_by_batch[b] = a_sb

    run_attention(0, qkv_tasks(1, ps_bg), bg_every=4)
    run_attention(1, proj_tasks(0), bg_every=12)
    for t in proj_tasks(1):
        t()

    ctx.close()


# ------------------------------------------------------------------- host side
def _host_prep(x, emb, gn_gamma, gn_beta, emb_w, emb_b, qkv_w, qkv_b,
               proj_w, proj_b):
    x = np.ascontiguousarray(np.asarray(x, np.float32).reshape(B, C, T))
    embT = np.ascontiguousarray(np.asarray(emb, np.float32).T)           # [E, B]

    qkv_wT = np.array(np.asarray(qkv_w, np.float32).T)                   # [C, 3C]
    wview = qkv_wT.reshape(C, NH, 3, CH)
    wview[:, :, 0, :] *= SCALE
    wview[:, :, 1, :] *= SCALE
    # permute cout: [q head 0..7 | k head 0..7 | v head 0..7] so pair/group
    # slices are contiguous for the PE stationary operand
    o = np.arange(3 * C).reshape(NH, 3, CH)
    perm = np.concatenate([o[:, 0, :].ravel(), o[:, 1, :].ravel(), o[:, 2, :].ravel()])
    qkv_wT_bf = np.ascontiguousarray(qkv_wT[:, perm].astype(ml_dtypes.bfloat16))

    emb_wT = np.ascontiguousarray(np.asarray(emb_w, np.float32).T.astype(ml_dtypes.bfloat16))  # [E, 2C]
    proj_wT = np.ascontiguousarray(np.asarray(proj_w, np.float32).T.astype(ml_dtypes.bfloat16))  # [C, C]

    gamma4 = np.ascontiguousarray(np.asarray(gn_gamma, np.float32).reshape(4, 128).T)
    beta4 = np.ascontiguousarray(np.asarray(gn_beta, np.float32).reshape(4, 128).T)

    p = np.arange(128)
    g = np.arange(8)
    mask_g = (p[:, None] // GSIZE == g[None, :]).astype(np.float32)      # [128, 8]
    mask_b = np.ascontiguousarray(mask_g.T)                              # [8, 128]

    qkv_b = np.asarray(qkv_b, np.float32)
    qb = qkv_b.reshape(NH, 3, CH)
    qkvb_qk = np.zeros((128, 8), np.float32)
    for hp in range(4):
        for par in range(2):
            qkvb_qk[64 * par:64 * par + 64, hp] = qb[2 * hp + par, 0] * SCALE
            qkvb_qk[64 * par:64 * par + 64, 4 + hp] = qb[2 * hp + par, 1] * SCALE

    emb_b2 = np.ascontiguousarray(
        np.broadcast_to(np.asarray(emb_b, np.float32)[None, :], (2, 2 * C))
    )

    # effective proj bias: proj_b + proj_w @ v_bias (v bias commutes with softmax)
    pb_eff = (np.asarray(proj_b, np.float64)
              + np.asarray(proj_w, np.float64) @ qb[:, 2, :].reshape(C).astype(np.float64))
    pb_eff = pb_eff.astype(np.float32)
    with_proj_bias = bool(np.any(pb_eff != 0))
    proj_b4 = np.ascontiguousarray(pb_eff.reshape(4, 128).T)

    shared = dict(qkv_wT=qkv_wT_bf, emb_wT=emb_wT, proj_wT=proj_wT,
                  gamma4=gamma4, beta4=beta4, mask_g=mask_g, mask_b=mask_b,
                  qkvb_qk=qkvb_qk, emb_b2=emb_b2)
    if with_proj_bias:
        shared["proj_b4"] = proj_b4

    with_qkv_bias = bool(np.any(qkvb_qk))
    in_maps = []
    for i in range(NCORES):
        m = dict(shared)
        m["x"] = np.ascontiguousarray(x[BLOC * i:BLOC * (i + 1)])
        m["embT"] = np.ascontiguousarray(embT[:, BLOC * i:BLOC * (i + 1)])
        in_maps.append(m)
    return in_maps, with_proj_bias, with_qkv_bias


_PROGRAM_CACHE = {}


def kernel(**inputs):
    global LAST_EXEC_NS, LAST_TRACE_DIR
    in_maps, with_proj_bias, with_qkv_bias = _host_prep(**inputs)
    key = (with_proj_bias, with_qkv_bias)
    if key not in _PROGRAM_CACHE:
        _PROGRAM_CACHE[key] = build_program(with_proj_bias, with_qkv_bias)
    nc = _PROGRAM_CACHE[key]

    kwargs = {}
    if TRACE:
        import tempfile
        _install_prof_shim()
        kwargs = dict(trace=True, tmpdir=tempfile.mkdtemp(prefix="attn_trace_"))
    res = run_bass_kernel_spmd(nc, in_maps, core_ids=list(range(NCORES)), **kwargs)
    LAST_EXEC_NS = res.exec_time_ns
    LAST_TRACE_DIR = kwargs.get("tmpdir")

    out = np.empty((B, C, T), np.float32)
    for i in range(NCORES):
        out[BLOC * i:BLOC * (i + 1)] = res.results[i]["out"]
    return out.reshape(B, C, HSP, WSP)


def _install_prof_shim():
    """Register the NTFF profile hook (missing antenv.axon_hooks in this image)."""
    import sys
    import types
    try:
        import antenv
        if not hasattr(antenv, "axon_hooks"):
            mod = types.ModuleType("antenv.axon_hooks")
            mod._hook = None
            mod.set_axon_ntff_profile_hook = lambda h: setattr(mod, "_hook", h)
            mod.get_axon_ntff_profile_hook = lambda: mod._hook
            sys.modules["antenv.axon_hooks"] = mod
            antenv.axon_hooks = mod
        from antenv.axon_hooks import (
            get_axon_ntff_profile_hook,
            set_axon_ntff_profile_hook,
        )
        if get_axon_ntff_profile_hook() is None:
            from trn_agent_boot.trn_boot import _ntff_profile_via_ctypes
            set_axon_ntff_profile_hook(
                _ntff_profile_via_ctypes("/opt/axon/libaxon_pjrt.so")
            )
        from concourse import bass_utils
        bass_utils.upload_artifacts = lambda tmpdir: "local://" + tmpdir
    except Exception as exc:  # profiling is best-effort
        print("prof shim install failed:", exc)
